# revision 27
# baseline (speedup 1.0000x reference)
"""Chamfer loss kernel for Trainium2, 8 NeuronCores.

ONE matmul per batch computes M[i,j] = g_i.p_j - |g_i|^2/2 - |p_j|^2/2
  = -P[i,j]/2  (P = squared pairwise distance), by augmenting both operand
blocks with ones rows and host-precomputed -|x|^2/2 rows. Then
  min_j P[i,:] = -2 max_j M[i,:]   and   min_i P[:,j] = -2 max_i M[:,j],
so one distance matrix serves both Chamfer directions (PE work halved vs
computing each direction's own matrix).

fp32 matmuls stream at 1/4 rate on the TRN2 PE, so each fp32 operand is
split hi/lo into fp16 (exact products, fp32 PSUM accumulation) and the
three cross terms ride along in the contraction dim: one K=13 fp16
matmul per tile-column runs at full PE rate with ~1e-6 abs error.

Per 128-row tile: matmuls (N=512) fill [128, 2048] PSUM strips; ScalarE
drains each strip to SBUF as fp16 (near the max, fp16 rounding is ~1e-6
abs since values there are tiny). VectorE then does
  - row maxes:  tensor_scalar(x*1, accum max) from fp16 SBUF -> 4x mode
  - col maxes:  in-place tensor_tensor max into a running [128, n] fp16
    buffer -> 2x_1p mode
The column direction ends with a 32x32 stream transpose + strided
reduce_max giving per-quadrant column maxes (DVE cannot cross 32-row
quadrants); the final 4-way max + sum lands in the host combine, like
the baseline's host-side partial sums.

stage_dk strips out of every 16 are instead drained by DVE tensor_copy
straight from PSUM to rebalance ScalarE vs VectorE load.

Sharding: batch dim 16 -> 2 per core; host gathers per-core partials.
"""

import sys

import numpy as np

sys.path.insert(0, "/opt/trn_rl_repo")

import concourse.bass as bass  # noqa: E402
import concourse.mybir as mybir  # noqa: E402
import concourse.tile as tile  # noqa: E402
from concourse import bacc  # noqa: E402
from concourse.bass_utils import run_bass_kernel_spmd  # noqa: E402

B, N_FULL, D = 16, 4096, 3
NCORES = 8
BLOC = B // NCORES  # batches per core
K = 13  # fp16 hi/lo split: 3x3 coord cross terms + 2x2 norm/ones rows
FREE = 512  # matmul free dim (one fp32 PSUM bank)

_built = {}


def build(n=N_FULL, bloc=BLOC, reps=1, stage_dk=0, sw=2048, ablate="", tail="st"):
    """Build the per-core Bass module.

    stage_dk: of every 16 strips, this many are DVE-copied out of PSUM
      (1x) instead of ScalarE-copied, to rebalance ACT vs DVE load.
    sw: strip width (PSUM drain granularity), multiple of FREE.
    ablate: comma-joined of {mm,act,ts,tt,tail} to skip components for
      HW timing ablations (results become wrong).
    tail: "st" = stream-transpose + strided reduce (all DVE);
          "dma" = xbar DMA transposes + per-chunk 4x accum max.
    """
    key = (n, bloc, reps, stage_dk, sw, ablate, tail)
    if key in _built:
        return _built[key]

    sw = min(sw, n)
    nt = n // 128  # 128-row tiles
    ns = n // sw  # strips per row tile
    nch = sw // FREE  # matmuls per strip

    abl = set(ablate.split(",")) if ablate else set()
    nc = bacc.Bacc("TRN2", target_bir_lowering=False, debug=False)
    fp32 = mybir.dt.float32
    fp16 = mybir.dt.float16

    gblkE = nc.dram_tensor("gblkE", [bloc, K, n], fp16, kind="ExternalInput")
    pblkE = nc.dram_tensor("pblkE", [bloc, K, n], fp16, kind="ExternalInput")
    ncols = (n // 128) if tail == "dma" else (n // 32)
    rows_dram = nc.dram_tensor("rows_out", [128, bloc], fp32, kind="ExternalOutput")
    cols_dram = nc.dram_tensor(
        "cols_out", [128, bloc * ncols], fp32, kind="ExternalOutput"
    )

    with tile.TileContext(nc) as tc:
        with (
            tc.tile_pool(name="blocks", bufs=1) as blocks,
            tc.tile_pool(name="small", bufs=1) as small,
            tc.tile_pool(name="stg", bufs=3) as stg,
            tc.tile_pool(name="scr", bufs=2) as scrp,
            tc.tile_pool(name="run", bufs=2) as runp,
            tc.tile_pool(name="rp", bufs=2) as rpp,
            tc.tile_pool(name="psum", bufs=2, space="PSUM") as psum_pool,
        ):
            rows_sb = small.tile([128, bloc], fp32, tag="rows_sb")
            cols_sb = small.tile([128, bloc * ncols], fp32, tag="cols_sb")

            gsb = []
            psb = []
            for b in range(bloc):
                g = blocks.tile([K, n], fp16, tag=f"g{b}")
                p = blocks.tile([K, n], fp16, tag=f"p{b}")
                nc.sync.dma_start(out=g[:], in_=gblkE[b])
                nc.sync.dma_start(out=p[:], in_=pblkE[b])
                gsb.append(g)
                psb.append(p)

            def emit_main():
                strip_ctr = 0
                for b in range(bloc):
                    runmax = runp.tile([128, n], fp16, tag="runmax")
                    rowpart = rpp.tile([128, nt * ns], fp32, tag="rowpart")
                    for t in range(nt):
                        staged = stg.tile([128, n], fp16, tag="staged")
                        for s in range(ns):
                            ps = psum_pool.tile([128, sw], fp32, tag="ps")
                            if "mm" not in abl:
                                for c in range(nch):
                                    j0 = s * sw + c * FREE
                                    nc.tensor.matmul(
                                        ps[:, c * FREE:(c + 1) * FREE],
                                        gsb[b][:, t * 128:(t + 1) * 128],
                                        psb[b][:, j0:j0 + FREE],
                                    )
                            sg = staged[:, s * sw:(s + 1) * sw]
                            direct = strip_ctr % 16 < stage_dk
                            strip_ctr += 1
                            if direct:
                                # DVE drains PSUM to fp16 (ACT relief)
                                nc.vector.tensor_copy(sg, ps[:])
                            elif "act" not in abl:
                                # ACT drains PSUM to fp16
                                nc.scalar.copy(sg, ps[:])
                            # DVE rowmax at 4x from the fp16 strip
                            if "ts" not in abl:
                                scr = scrp.tile([128, sw], fp16, tag="scr")
                                nc.vector.tensor_scalar(
                                    out=scr[:], in0=sg, scalar1=1.0,
                                    scalar2=None, op0=mybir.AluOpType.mult,
                                    op1=mybir.AluOpType.max,
                                    accum_out=rowpart[:, t * ns + s:t * ns + s + 1],
                                )
                            # running col max at 2x_1p
                            if "tt" not in abl:
                                rg = runmax[:, s * sw:(s + 1) * sw]
                                if t == 0:
                                    nc.vector.tensor_copy(rg, sg)
                                else:
                                    nc.vector.tensor_tensor(
                                        out=rg, in0=sg, in1=rg,
                                        op=mybir.AluOpType.max,
                                    )
                    # rows: max over strips within each tile, sum over tiles
                    if "ts" not in abl:
                        if ns > 1:
                            rowtile = rpp.tile([128, nt], fp32, tag="rowtile")
                            nc.vector.reduce_max(
                                rowtile[:],
                                rowpart[:].rearrange("p (t s) -> p t s", s=ns),
                                axis=mybir.AxisListType.X,
                            )
                        else:
                            rowtile = rowpart
                        nc.vector.reduce_sum(
                            rows_sb[:, b:b + 1], rowtile[:],
                            axis=mybir.AxisListType.X,
                        )
                    if "tail" in abl:
                        continue
                    if tail == "dma":
                        # xbar DMA transposes 128-column chunks of runmax;
                        # a 4x tensor_scalar accum-max then finishes the
                        # partition max per chunk: full col maxes on device.
                        nck = n // 128
                        tp = runp.tile([128, n], fp16, tag="tp")
                        for c in range(nck):
                            nc.sync.dma_start_transpose(
                                tp[:, c * 128:(c + 1) * 128],
                                runmax[:, c * 128:(c + 1) * 128],
                            )
                        cmscr = scrp.tile([128, n], fp16, tag="cmscr")
                        for c in range(nck):
                            nc.vector.tensor_scalar(
                                out=cmscr[:, c * 128:(c + 1) * 128],
                                in0=tp[:, c * 128:(c + 1) * 128],
                                scalar1=1.0, scalar2=None,
                                op0=mybir.AluOpType.mult,
                                op1=mybir.AluOpType.max,
                                accum_out=cols_sb[:, b * ncols + c:b * ncols + c + 1],
                            )
                    else:
                        # cols: 32x32 stream transpose puts each quadrant's
                        # partitions into the free dim; strided reduce_max
                        # yields per-quadrant column maxes cm[32A+c, B] =
                        # max over quadrant-A partitions of column j=32B+c.
                        # (DVE ops cannot cross 32-partition quadrants, so
                        # the final 4-way max + sum finishes on the host.)
                        ctr = scrp.tile([128, n], fp16, tag="ctr")
                        nc.vector.transpose(ctr[:], runmax[:])
                        nc.vector.reduce_max(
                            cols_sb[:, b * ncols:(b + 1) * ncols],
                            ctr[:].rearrange("p (B r) -> p B r", r=32),
                            axis=mybir.AxisListType.X,
                        )

            if reps == 1:
                emit_main()
            else:
                with tc.For_i(0, reps, 1):
                    emit_main()

            nc.sync.dma_start(out=rows_dram[:], in_=rows_sb[:])
            nc.sync.dma_start(out=cols_dram[:], in_=cols_sb[:])

    nc.compile()
    _built[key] = (nc, "gblkE", "pblkE", "rows_out", "cols_out")
    return _built[key]


def make_blocks(x, role):
    """[bloc, n, 3] coords -> [bloc, 13, n] fp16 hi/lo-split block.

    Each fp32 value v is split as v = hi + lo with hi = fp16(v),
    lo = fp16(v - hi); the matmul computes hi*hi + hi*lo + lo*hi (the
    lo*lo term, ~2^-22 relative, is dropped). Row pairing (k: lhsT x rhs):
      0-2:  ghi_c  x phi_c      3-5:  ghi_c x plo_c     6-8: glo_c x phi_c
      9-10: 1 x [pn_hi, pn_lo]  11-12: [gn_hi, gn_lo] x 1
    so M[i,j] = g_i.p_j - |g_i|^2/2 - |p_j|^2/2 to ~1e-6 abs.
    """
    x = np.asarray(x, dtype=np.float32)
    bloc, n, _ = x.shape
    chi = x.astype(np.float16)
    clo = (x - chi.astype(np.float32)).astype(np.float16)
    chi = chi.transpose(0, 2, 1)  # [bloc, 3, n]
    clo = clo.transpose(0, 2, 1)
    nrm = (-0.5 * (x.astype(np.float64) ** 2).sum(-1)).astype(np.float32)
    nhi = nrm.astype(np.float16)
    nlo = (nrm - nhi.astype(np.float32)).astype(np.float16)
    out = np.empty((bloc, K, n), dtype=np.float16)
    if role == "lhsT":
        out[:, 0:3] = chi
        out[:, 3:6] = chi
        out[:, 6:9] = clo
        out[:, 9] = 1.0
        out[:, 10] = 1.0
        out[:, 11] = nhi
        out[:, 12] = nlo
    else:
        out[:, 0:3] = chi
        out[:, 3:6] = clo
        out[:, 6:9] = chi
        out[:, 9] = nhi
        out[:, 10] = nlo
        out[:, 11] = 1.0
        out[:, 12] = 1.0
    return np.ascontiguousarray(out)


def shard_inputs(preds, gts, bloc=BLOC, ncores=NCORES):
    preds = np.asarray(preds, dtype=np.float32)
    gts = np.asarray(gts, dtype=np.float32)
    in_maps = []
    for c in range(ncores):
        lo = c * bloc
        in_maps.append({
            "gblkE": make_blocks(gts[lo:lo + bloc], "lhsT"),
            "pblkE": make_blocks(preds[lo:lo + bloc], "rhs"),
        })
    return in_maps


def combine_outputs(rows, cols, n=N_FULL, b=B, tail="st"):
    tot = np.sum([r.astype(np.float64).sum() for r in rows])
    for c in cols:
        if tail == "dma":
            # entries are already full column maxes
            tot += c.astype(np.float64).sum()
        else:
            # c: [128, bloc * n/32]; rows 32A+cc hold quadrant-A col maxes
            cm = c.astype(np.float64).reshape(4, 32, -1)
            tot += cm.max(axis=0).sum()
    return np.float32(-2.0 * tot / (b * n))


BEST_DK = 0
BEST_TAIL = "st"


def kernel(preds, gts):
    nc, _, _, o1, o2 = build(stage_dk=BEST_DK, tail=BEST_TAIL)
    in_maps = shard_inputs(preds, gts)
    res = run_bass_kernel_spmd(nc, in_maps, core_ids=list(range(NCORES)))
    return combine_outputs(
        [r[o1] for r in res.results], [r[o2] for r in res.results],
        tail=BEST_TAIL,
    )


def _numpy_chamfer(preds, gts):
    tot = 0.0
    for b_ in range(preds.shape[0]):
        gg = (gts[b_] ** 2).sum(-1)
        pp = (preds[b_] ** 2).sum(-1)
        zz = gts[b_] @ preds[b_].T
        P = gg[:, None] + pp[None, :] - 2 * zz
        tot += P.min(axis=0).mean() + P.min(axis=1).mean()
    return tot / preds.shape[0]


if __name__ == "__main__":
    from concourse.bass_interp import CoreSim

    n = int(sys.argv[1]) if len(sys.argv) > 1 else 512
    bloc = int(sys.argv[2]) if len(sys.argv) > 2 else 1
    dk = int(sys.argv[3]) if len(sys.argv) > 3 else 0
    nc, gn, pn, o1, o2 = build(n=n, bloc=bloc, stage_dk=dk)
    rng = np.random.default_rng(0)
    preds = rng.standard_normal((bloc, n, D)).astype(np.float32)
    gts = rng.standard_normal((bloc, n, D)).astype(np.float32)

    sim = CoreSim(nc)
    sim.tensor(gn)[:] = make_blocks(gts, "lhsT")
    sim.tensor(pn)[:] = make_blocks(preds, "rhs")
    sim.simulate()
    got = combine_outputs([sim.tensor(o1)], [sim.tensor(o2)], n=n, b=bloc)
    want = _numpy_chamfer(preds, gts)
    print("sim:", got, "numpy:", want, "rel err:", abs(got - want) / abs(want))
